# revision 25
# baseline (speedup 1.0000x reference)
"""Non-local (spatial self-attention) denoising block on 8 Trainium2 cores.

Reference math (per sample n, with x:[C,HW], D=C/2):
    t = (W_theta @ x + b_theta) / sqrt(D)      [D, HW]   (1/sqrt(D) folded in)
    p = W_phi   @ x + b_phi                    [D, HW]
    S[q,k] = t[:,q] . p[:,k]
    f = softmax_k(S)
    attn = x @ f.T  (i.e. attn[c,q] = sum_k f[q,k] x[c,k])
    out = x + W_fuse @ attn + b_fuse

Device formulation (all matmuls bf16 with fp32 PSUM accumulation):
    Sᵀ[k,q] = p.T @ t  -- keys on partitions so no transposes are needed
    e = exp(Sᵀ)        -- no max subtraction (|S| <= ~8 by construction)
    G'ᵀ = xᵀ @ W_fuseᵀ -- fuses the output conv into the values: [HW, C]
    Zb = onesᵀ @ e     -- Z[q] replicated on all 128 partitions (sum+broadcast)
    y = G'ᵀᵀ @ e       -- [C, HW] unnormalized
    out = y * (1/Zb) + (x + b_fuse)   (division commutes through the conv)

Sharding: data-parallel over batch N=32 -> 4 samples per core on 8 cores.
"""

import numpy as np
import ml_dtypes

import concourse.bass as bass
import concourse.tile as tile
from concourse import bacc, mybir
from concourse import bass_utils

F32 = mybir.dt.float32
BF16 = mybir.dt.bfloat16
AF = mybir.ActivationFunctionType

N, C, H, W = 32, 512, 32, 32
D = C // 2
HW = H * W
NCORES = 8
NS = N // NCORES  # samples per core
P = 128
CT = C // P   # 4 c-tiles
KT = HW // P  # 8 hw-tiles
MT_D = (2 * D) // P  # 4 m-tiles of combined theta/phi conv
NQ = HW // 512  # 2 free-dim halves


def _emit(tc):
    nc = tc.nc

    x_bf = nc.dram_tensor("x_bf", [NS, C, HW], BF16, kind="ExternalInput").ap()
    # residual (x + b_fuse), pre-transposed to [HW, C] on the host
    x_res = nc.dram_tensor("x_res", [NS, HW, C], F32, kind="ExternalInput").ap()
    wcat_t = nc.dram_tensor("wcat_t", [C, 2 * D], BF16, kind="ExternalInput").ap()
    b_cat = nc.dram_tensor("b_cat", [2 * D, 1], F32, kind="ExternalInput").ap()
    wfu_t = nc.dram_tensor("wfu_t", [C, C], BF16, kind="ExternalInput").ap()
    # output is produced transposed: [HW, C]; host transposes back
    out_d = nc.dram_tensor("out", [NS, HW, C], F32, kind="ExternalOutput").ap()

    import contextlib
    ctx = contextlib.ExitStack()
    with ctx:
        # ---- constant pools ----
        wpool = ctx.enter_context(tc.tile_pool(name="wpool", bufs=1))
        # weights: [c, *] chunked by 128 c-rows along free dim.
        # wcat + x(sample 0) gate the first conv matmuls: split them into
        # per-c-chunk DMAs on two queues so matmul k-steps start as chunks land.
        wcat_sb = wpool.tile([P, CT * 2 * D], BF16)

        # ---- working pools ----
        xbf_pool = ctx.enter_context(tc.tile_pool(name="xbf", bufs=2))
        xres_pool = ctx.enter_context(tc.tile_pool(name="xres", bufs=2))
        tp_pool = ctx.enter_context(tc.tile_pool(name="tp", bufs=2))
        gt_pool = ctx.enter_context(tc.tile_pool(name="gt", bufs=2))
        e_pool = ctx.enter_context(tc.tile_pool(name="e", bufs=2))
        rz_pool = ctx.enter_context(tc.tile_pool(name="rz", bufs=2))
        fin_pool = ctx.enter_context(tc.tile_pool(name="fin", bufs=3))
        out_pool = ctx.enter_context(tc.tile_pool(name="outp", bufs=3))

        psum_mm = ctx.enter_context(tc.tile_pool(name="psmm", bufs=2, space="PSUM"))
        psum_g = ctx.enter_context(tc.tile_pool(name="psg", bufs=4, space="PSUM"))
        esum_pool = ctx.enter_context(tc.tile_pool(name="esum", bufs=1))

        # HAM pre-warm: ~3.4us of junk matmuls during the initial DMA wait
        # so the real matmuls start at 2.4 GHz instead of 1.2 GHz.
        ones_sb = wpool.tile([P, P], F32)
        nc.vector.memset(ones_sb[:], 1.0)
        ones_bf = wpool.tile([P, P], BF16)
        nc.vector.memset(ones_bf[:], 1.0)
        ps_warm = psum_g.tile([P, P], F32, tag="g", name="ps_warm")
        for w in range(12):
            nc.tensor.matmul(ps_warm[:], ones_bf[:], ones_bf[:],
                             start=True, stop=True)

        xbf_tiles = {}
        xbf_tiles[0] = xbf_pool.tile([P, CT * HW], BF16, tag="xbf", name="xbf0")
        for k in range(CT):
            nc.sync.dma_start(
                wcat_sb[:, k * 2 * D:(k + 1) * 2 * D],
                wcat_t.rearrange("(t p) d -> t p d", p=P)[k],
            )
            nc.gpsimd.dma_start(
                xbf_tiles[0][:, k * HW:(k + 1) * HW],
                x_bf[0].rearrange("(t p) f -> t p f", p=P)[k],
            )

        # remaining constants (not needed by the first conv matmuls)
        wfu_sb = wpool.tile([P, CT * C], BF16)
        nc.sync.dma_start(
            wfu_sb.rearrange("p (t d) -> p t d", d=C),
            wfu_t.rearrange("(t p) d -> p t d", p=P),
        )
        bcat_sb = wpool.tile([P, MT_D], F32)
        nc.sync.dma_start(
            bcat_sb.rearrange("p (t o) -> p t o", o=1),
            b_cat.rearrange("(t p) o -> p t o", p=P),
        )
        for s in range(NS):
            # ---- load x (bf16 for matmuls, f32 residual w/ b_fuse folded) ----
            if s not in xbf_tiles:
                xbf_tiles[s] = xbf_pool.tile(
                    [P, CT * HW], BF16, tag="xbf", name=f"xbf{s}"
                )
                nc.sync.dma_start(
                    xbf_tiles[s].rearrange("p (t f) -> p t f", f=HW),
                    x_bf[s].rearrange("(t p) f -> p t f", p=P),
                )
            xbf_sb = xbf_tiles[s]
            xres_sb = xres_pool.tile([P, KT * C], F32, tag="xres")
            nc.sync.dma_start(
                xres_sb.rearrange("p (t f) -> p t f", f=C),
                x_res[s].rearrange("(t p) f -> p t f", p=P),
            )

            # ---- combined theta/phi 1x1 conv: tp = wcat.T @ x + b ----
            # tp_sb chunks m=0,1 -> theta [256, HW]; m=2,3 -> phi
            tp_sb = tp_pool.tile([P, MT_D * HW], BF16, tag="tp")
            for m in range(MT_D):
                ps_cv = psum_mm.tile([P, HW], F32, tag="mm", name=f"ps_cv{s}_{m}")
                for k in range(CT):
                    for nq in range(NQ):
                        nc.tensor.matmul(
                            ps_cv[:, nq * 512:(nq + 1) * 512],
                            wcat_sb[:, k * 2 * D + m * P: k * 2 * D + (m + 1) * P],
                            xbf_sb[:, k * HW + nq * 512: k * HW + nq * 512 + 512],
                            start=(k == 0),
                            stop=(k == CT - 1),
                        )
                nc.scalar.activation(
                    tp_sb[:, m * HW:(m + 1) * HW],
                    ps_cv[:],
                    AF.Identity,
                    bias=bcat_sb[:, m:m + 1],
                )

            # ---- G'T = x.T @ wfu.T : [HW, C], fused-values ----
            gt_sb = gt_pool.tile([P, KT * C], BF16, tag="gt")
            for m in range(KT):
                ps_g = psum_g.tile([P, C], F32, tag="g", name=f"ps_g{s}_{m}")
                for k in range(CT):
                    nc.tensor.matmul(
                        ps_g[:],
                        xbf_sb[:, k * HW + m * P: k * HW + (m + 1) * P],
                        wfu_sb[:, k * C:(k + 1) * C],
                        start=(k == 0),
                        stop=(k == CT - 1),
                    )
                nc.scalar.activation(
                    gt_sb[:, m * C:(m + 1) * C], ps_g[:], AF.Copy,
                )

            # ---- S^T = p.T @ t ; e = exp(S^T) ----
            # Z via add-tree over the 8 e-tiles (engine ALUs are fp32-internal,
            # so f32 outputs make the partial sums exact), then one ones-matmul
            # both sums over the 128 partitions and broadcasts Z to them.
            e_sb = e_pool.tile([P, KT * HW], BF16, tag="e")
            ea_sb = esum_pool.tile([P, 4 * HW], F32, tag="ea")
            eb_sb = esum_pool.tile([P, 2 * HW], F32, tag="eb")
            es_sb = esum_pool.tile([P, HW], F32, tag="es")
            for m in range(KT):
                ps_s = psum_mm.tile([P, HW], F32, tag="mm", name=f"ps_s{s}_{m}")
                for kd in range(2):
                    for nq in range(NQ):
                        nc.tensor.matmul(
                            ps_s[:, nq * 512:(nq + 1) * 512],
                            tp_sb[:, (2 + kd) * HW + m * P: (2 + kd) * HW + (m + 1) * P],
                            tp_sb[:, kd * HW + nq * 512: kd * HW + nq * 512 + 512],
                            start=(kd == 0),
                            stop=(kd == 1),
                        )
                nc.scalar.activation(
                    e_sb[:, m * HW:(m + 1) * HW], ps_s[:], AF.Exp,
                )
                if m % 2 == 1:
                    # level-0 pair adds alternate GpSimd / DVE
                    j = m // 2
                    eng = nc.gpsimd if j % 2 == 0 else nc.vector
                    eng.tensor_add(
                        ea_sb[:, j * HW:(j + 1) * HW],
                        e_sb[:, (m - 1) * HW: m * HW],
                        e_sb[:, m * HW:(m + 1) * HW],
                    )
                    if m % 4 == 3:
                        jj = m // 4
                        nc.vector.tensor_add(
                            eb_sb[:, jj * HW:(jj + 1) * HW],
                            ea_sb[:, (jj * 2) * HW:(jj * 2 + 1) * HW],
                            ea_sb[:, (jj * 2 + 1) * HW:(jj * 2 + 2) * HW],
                        )
            nc.vector.tensor_add(es_sb[:], eb_sb[:, 0:HW], eb_sb[:, HW:2 * HW])

            # ---- yT = e.T @ G'T : [HW, C] tiles, q on partitions ----
            # With q on partitions, 1/Z is a per-partition scalar and fuses
            # into the mandatory ACT psum->sbuf copy (activation scale AP).
            # PE order: yT0, yT1, Zc, yT2..7 -- Zc depends on the add-tree,
            # placed 2 tiles deep to hide its latency.
            def yt_mmtile(m):
                ps_yt = psum_g.tile([P, C], F32, tag="g", name=f"ps_yt{s}_{m}")
                for k in range(KT):
                    nc.tensor.matmul(
                        ps_yt[:],
                        e_sb[:, k * HW + m * P: k * HW + (m + 1) * P],
                        gt_sb[:, k * C:(k + 1) * C],
                        start=(k == 0),
                        stop=(k == KT - 1),
                    )
                return ps_yt

            ps_yts = [yt_mmtile(0), yt_mmtile(1)]

            # Zc[q] compact: stationary es-slice x ones -> [128, 8] per q-tile
            # (out column j all equal Z[q]); single strided reciprocal after.
            ps_zc = psum_g.tile([P, KT * 8], F32, tag="g", name=f"ps_zc{s}")
            for m in range(KT):
                nc.tensor.matmul(
                    ps_zc[:, m * 8:(m + 1) * 8],
                    es_sb[:, m * P:(m + 1) * P],
                    ones_sb[:, 0:8],
                    start=True,
                    stop=True,
                )
            rc_sb = rz_pool.tile([P, KT], F32, tag="rz")
            nc.vector.reciprocal_approx_fast(
                out=rc_sb[:],
                in_=ps_zc.rearrange("p (m j) -> p m j", j=8)[:, :, 0],
            )

            ps_yts += [yt_mmtile(m) for m in range(2, KT)]

            # normalize (ACT copy w/ per-partition scale) + residual + store
            for m in range(KT):
                t1 = fin_pool.tile([P, C], F32, tag="fin", name=f"t1_{s}_{m}")
                nc.scalar.activation(
                    t1[:], ps_yts[m][:], AF.Identity,
                    scale=rc_sb[:, m:m + 1],
                )
                o_sb = out_pool.tile([P, C], F32, tag="o", name=f"o_{s}_{m}")
                nc.vector.tensor_add(
                    o_sb[:], t1[:], xres_sb[:, m * C:(m + 1) * C],
                )
                nc.sync.dma_start(
                    out_d[s].rearrange("(t p) f -> t p f", p=P)[m],
                    o_sb[:],
                )


_CACHE = {}


def _build():
    if "nc" not in _CACHE:
        nc = bacc.Bacc("TRN2", target_bir_lowering=False, debug=False)
        with tile.TileContext(nc) as tc:
            _emit(tc)
        nc.compile()
        _CACHE["nc"] = nc
    return _CACHE["nc"]


def _prep_in_maps(x, W_theta, b_theta, W_phi, b_phi, W_fuse, b_fuse):
    bf = ml_dtypes.bfloat16
    scale = np.float32(1.0 / np.sqrt(np.float32(D)))
    xf = np.ascontiguousarray(np.asarray(x, dtype=np.float32).reshape(N, C, HW))
    x_bf = xf.astype(bf)
    # residual (x + b_fuse), transposed to [HW, C] to match the yT layout
    x_res = np.ascontiguousarray(
        (xf + np.asarray(b_fuse, dtype=np.float32)[None, :, None]).transpose(0, 2, 1)
    )
    wcat_t = np.ascontiguousarray(
        np.concatenate([W_theta.astype(np.float32) * scale,
                        W_phi.astype(np.float32)], axis=0).T
    ).astype(bf)
    b_cat = np.concatenate([b_theta.astype(np.float32) * scale,
                            b_phi.astype(np.float32)]).reshape(2 * D, 1)
    wfu_t = np.ascontiguousarray(W_fuse.astype(np.float32).T).astype(bf)

    in_maps = []
    for c in range(NCORES):
        sl = slice(c * NS, (c + 1) * NS)
        in_maps.append({
            "x_bf": np.ascontiguousarray(x_bf[sl]),
            "x_res": np.ascontiguousarray(x_res[sl]),
            "wcat_t": wcat_t,
            "b_cat": b_cat.astype(np.float32),
            "wfu_t": wfu_t,
        })
    return in_maps


def _run(inputs, trace=False, **kw):
    nc = _build()
    in_maps = _prep_in_maps(**inputs)
    res = bass_utils.run_bass_kernel_spmd(
        nc, in_maps, core_ids=list(range(NCORES)), trace=trace, **kw
    )
    out = np.concatenate([res.results[c]["out"] for c in range(NCORES)], axis=0)
    # device produced [NS, HW, C]; transpose back to [N, C, H, W]
    out = out.transpose(0, 2, 1).reshape(N, C, H, W)
    return np.ascontiguousarray(out, dtype=np.float32), res


def kernel(**inputs):
    out, _ = _run(inputs, trace=False)
    return out


# revision 26
# speedup vs baseline: 1.0213x; 1.0213x over previous
"""Non-local (spatial self-attention) denoising block on 8 Trainium2 cores.

Reference math (per sample n, with x:[C,HW], D=C/2):
    t = (W_theta @ x + b_theta) / sqrt(D)      [D, HW]   (1/sqrt(D) folded in)
    p = W_phi   @ x + b_phi                    [D, HW]
    S[q,k] = t[:,q] . p[:,k]
    f = softmax_k(S)
    attn = x @ f.T  (i.e. attn[c,q] = sum_k f[q,k] x[c,k])
    out = x + W_fuse @ attn + b_fuse

Device formulation (all matmuls bf16 with fp32 PSUM accumulation):
    Sᵀ[k,q] = p.T @ t  -- keys on partitions so no transposes are needed
    e = exp(Sᵀ)        -- no max subtraction (|S| <= ~8 by construction)
    G'ᵀ = xᵀ @ W_fuseᵀ -- fuses the output conv into the values: [HW, C]
    Zb = onesᵀ @ e     -- Z[q] replicated on all 128 partitions (sum+broadcast)
    y = G'ᵀᵀ @ e       -- [C, HW] unnormalized
    out = y * (1/Zb) + (x + b_fuse)   (division commutes through the conv)

Sharding: data-parallel over batch N=32 -> 4 samples per core on 8 cores.
"""

import numpy as np
import ml_dtypes

import concourse.bass as bass
import concourse.tile as tile
from concourse import bacc, mybir
from concourse import bass_utils

F32 = mybir.dt.float32
BF16 = mybir.dt.bfloat16
AF = mybir.ActivationFunctionType

N, C, H, W = 32, 512, 32, 32
D = C // 2
HW = H * W
NCORES = 8
NS = N // NCORES  # samples per core
P = 128
CT = C // P   # 4 c-tiles
KT = HW // P  # 8 hw-tiles
MT_D = (2 * D) // P  # 4 m-tiles of combined theta/phi conv
NQ = HW // 512  # 2 free-dim halves


def _emit(tc):
    nc = tc.nc

    x_bf = nc.dram_tensor("x_bf", [NS, C, HW], BF16, kind="ExternalInput").ap()
    # residual (x + b_fuse), pre-transposed to [HW, C] on the host
    x_res = nc.dram_tensor("x_res", [NS, HW, C], F32, kind="ExternalInput").ap()
    wcat_t = nc.dram_tensor("wcat_t", [C, 2 * D], BF16, kind="ExternalInput").ap()
    b_cat = nc.dram_tensor("b_cat", [2 * D, 1], F32, kind="ExternalInput").ap()
    wfu_t = nc.dram_tensor("wfu_t", [C, C], BF16, kind="ExternalInput").ap()
    # output is produced transposed: [HW, C]; host transposes back
    out_d = nc.dram_tensor("out", [NS, HW, C], F32, kind="ExternalOutput").ap()

    import contextlib
    ctx = contextlib.ExitStack()
    with ctx:
        # ---- constant pools ----
        wpool = ctx.enter_context(tc.tile_pool(name="wpool", bufs=1))
        # weights: [c, *] chunked by 128 c-rows along free dim.
        # wcat + x(sample 0) gate the first conv matmuls: split them into
        # per-c-chunk DMAs on two queues so matmul k-steps start as chunks land.
        wcat_sb = wpool.tile([P, CT * 2 * D], BF16)

        # ---- working pools ----
        xbf_pool = ctx.enter_context(tc.tile_pool(name="xbf", bufs=2))
        xres_pool = ctx.enter_context(tc.tile_pool(name="xres", bufs=2))
        tp_pool = ctx.enter_context(tc.tile_pool(name="tp", bufs=2))
        gt_pool = ctx.enter_context(tc.tile_pool(name="gt", bufs=2))
        e_pool = ctx.enter_context(tc.tile_pool(name="e", bufs=2))
        rz_pool = ctx.enter_context(tc.tile_pool(name="rz", bufs=2))
        fin_pool = ctx.enter_context(tc.tile_pool(name="fin", bufs=3))
        out_pool = ctx.enter_context(tc.tile_pool(name="outp", bufs=3))

        psum_mm = ctx.enter_context(tc.tile_pool(name="psmm", bufs=2, space="PSUM"))
        psum_g = ctx.enter_context(tc.tile_pool(name="psg", bufs=4, space="PSUM"))
        esum_pool = ctx.enter_context(tc.tile_pool(name="esum", bufs=1))

        # HAM pre-warm: ~3.4us of junk matmuls during the initial DMA wait
        # so the real matmuls start at 2.4 GHz instead of 1.2 GHz.
        ones_sb = wpool.tile([P, P], F32)
        nc.vector.memset(ones_sb[:], 1.0)
        ones_bf = wpool.tile([P, P], BF16)
        nc.vector.memset(ones_bf[:], 1.0)
        ps_warm = psum_g.tile([P, P], F32, tag="g", name="ps_warm")
        for w in range(12):
            nc.tensor.matmul(ps_warm[:], ones_bf[:], ones_bf[:],
                             start=True, stop=True)

        xbf_tiles = {}
        xbf_tiles[0] = xbf_pool.tile([P, CT * HW], BF16, tag="xbf", name="xbf0")
        for k in range(CT):
            nc.sync.dma_start(
                wcat_sb[:, k * 2 * D:(k + 1) * 2 * D],
                wcat_t.rearrange("(t p) d -> t p d", p=P)[k],
            )
            nc.gpsimd.dma_start(
                xbf_tiles[0][:, k * HW:(k + 1) * HW],
                x_bf[0].rearrange("(t p) f -> t p f", p=P)[k],
            )

        # remaining constants (not needed by the first conv matmuls)
        wfu_sb = wpool.tile([P, CT * C], BF16)
        nc.sync.dma_start(
            wfu_sb.rearrange("p (t d) -> p t d", d=C),
            wfu_t.rearrange("(t p) d -> p t d", p=P),
        )
        bcat_sb = wpool.tile([P, MT_D], F32)
        nc.sync.dma_start(
            bcat_sb.rearrange("p (t o) -> p t o", o=1),
            b_cat.rearrange("(t p) o -> p t o", p=P),
        )
        for s in range(NS):
            # ---- load x (bf16 for matmuls, f32 residual w/ b_fuse folded) ----
            if s not in xbf_tiles:
                xbf_tiles[s] = xbf_pool.tile(
                    [P, CT * HW], BF16, tag="xbf", name=f"xbf{s}"
                )
                nc.sync.dma_start(
                    xbf_tiles[s].rearrange("p (t f) -> p t f", f=HW),
                    x_bf[s].rearrange("(t p) f -> p t f", p=P),
                )
            xbf_sb = xbf_tiles[s]
            xres_sb = xres_pool.tile([P, KT * C], F32, tag="xres")
            nc.sync.dma_start(
                xres_sb.rearrange("p (t f) -> p t f", f=C),
                x_res[s].rearrange("(t p) f -> p t f", p=P),
            )

            # ---- combined theta/phi 1x1 conv: tp = wcat.T @ x + b ----
            # tp_sb chunks m=0,1 -> theta [256, HW]; m=2,3 -> phi
            tp_sb = tp_pool.tile([P, MT_D * HW], BF16, tag="tp")
            for m in range(MT_D):
                ps_cv = psum_mm.tile([P, HW], F32, tag="mm", name=f"ps_cv{s}_{m}")
                for k in range(CT):
                    for nq in range(NQ):
                        nc.tensor.matmul(
                            ps_cv[:, nq * 512:(nq + 1) * 512],
                            wcat_sb[:, k * 2 * D + m * P: k * 2 * D + (m + 1) * P],
                            xbf_sb[:, k * HW + nq * 512: k * HW + nq * 512 + 512],
                            start=(k == 0),
                            stop=(k == CT - 1),
                        )
                nc.scalar.activation(
                    tp_sb[:, m * HW:(m + 1) * HW],
                    ps_cv[:],
                    AF.Identity,
                    bias=bcat_sb[:, m:m + 1],
                )

            # ---- G'T = x.T @ wfu.T : [HW, C], fused-values ----
            gt_sb = gt_pool.tile([P, KT * C], BF16, tag="gt")
            for m in range(KT):
                ps_g = psum_g.tile([P, C], F32, tag="g", name=f"ps_g{s}_{m}")
                for k in range(CT):
                    nc.tensor.matmul(
                        ps_g[:],
                        xbf_sb[:, k * HW + m * P: k * HW + (m + 1) * P],
                        wfu_sb[:, k * C:(k + 1) * C],
                        start=(k == 0),
                        stop=(k == CT - 1),
                    )
                nc.scalar.activation(
                    gt_sb[:, m * C:(m + 1) * C], ps_g[:], AF.Copy,
                )

            # ---- S^T = p.T @ t ; e = exp(S^T) ----
            # Z via add-tree over the 8 e-tiles (engine ALUs are fp32-internal,
            # so f32 outputs make the partial sums exact), then one ones-matmul
            # both sums over the 128 partitions and broadcasts Z to them.
            e_sb = e_pool.tile([P, KT * HW], BF16, tag="e")
            ea_sb = esum_pool.tile([P, 4 * HW], F32, tag="ea")
            eb_sb = esum_pool.tile([P, 2 * HW], F32, tag="eb")
            es_sb = esum_pool.tile([P, HW], F32, tag="es")
            for m in range(KT):
                ps_s = psum_mm.tile([P, HW], F32, tag="mm", name=f"ps_s{s}_{m}")
                for kd in range(2):
                    for nq in range(NQ):
                        nc.tensor.matmul(
                            ps_s[:, nq * 512:(nq + 1) * 512],
                            tp_sb[:, (2 + kd) * HW + m * P: (2 + kd) * HW + (m + 1) * P],
                            tp_sb[:, kd * HW + nq * 512: kd * HW + nq * 512 + 512],
                            start=(kd == 0),
                            stop=(kd == 1),
                        )
                nc.scalar.activation(
                    e_sb[:, m * HW:(m + 1) * HW], ps_s[:], AF.Exp,
                )
                if m % 2 == 1:
                    # level-0 pair adds alternate GpSimd / DVE
                    j = m // 2
                    eng = nc.gpsimd if j % 2 == 0 else nc.vector
                    eng.tensor_add(
                        ea_sb[:, j * HW:(j + 1) * HW],
                        e_sb[:, (m - 1) * HW: m * HW],
                        e_sb[:, m * HW:(m + 1) * HW],
                    )
                    if m % 4 == 3:
                        jj = m // 4
                        nc.vector.tensor_add(
                            eb_sb[:, jj * HW:(jj + 1) * HW],
                            ea_sb[:, (jj * 2) * HW:(jj * 2 + 1) * HW],
                            ea_sb[:, (jj * 2 + 1) * HW:(jj * 2 + 2) * HW],
                        )
            nc.vector.tensor_add(es_sb[:], eb_sb[:, 0:HW], eb_sb[:, HW:2 * HW])

            # ---- yT = e.T @ G'T : [HW, C] tiles, q on partitions ----
            # With q on partitions, 1/Z is a per-partition scalar and fuses
            # into the mandatory ACT psum->sbuf copy (activation scale AP).
            # PE order: yT0, yT1, Zc, yT2..7 -- Zc depends on the add-tree,
            # placed 2 tiles deep to hide its latency.
            def yt_mmtile(m):
                ps_yt = psum_g.tile([P, C], F32, tag="g", name=f"ps_yt{s}_{m}")
                for k in range(KT):
                    nc.tensor.matmul(
                        ps_yt[:],
                        e_sb[:, k * HW + m * P: k * HW + (m + 1) * P],
                        gt_sb[:, k * C:(k + 1) * C],
                        start=(k == 0),
                        stop=(k == KT - 1),
                    )
                return ps_yt

            ps_yts = [yt_mmtile(0), yt_mmtile(1), yt_mmtile(2), yt_mmtile(3)]

            # Zc[q] compact: stationary es-slice x ones -> [128, 8] per q-tile
            # (out column j all equal Z[q]); single strided reciprocal after.
            # Lives in the (currently idle) psum_mm pool; placed 4 yt-tiles
            # deep so the add-tree has finished by the time the PE gets here.
            ps_zc = psum_mm.tile([P, KT * 8], F32, tag="mm", name=f"ps_zc{s}")
            for m in range(KT):
                nc.tensor.matmul(
                    ps_zc[:, m * 8:(m + 1) * 8],
                    es_sb[:, m * P:(m + 1) * P],
                    ones_sb[:, 0:8],
                    start=True,
                    stop=True,
                )
            rc_sb = rz_pool.tile([P, KT], F32, tag="rz")
            nc.vector.reciprocal_approx_fast(
                out=rc_sb[:],
                in_=ps_zc.rearrange("p (m j) -> p m j", j=8)[:, :, 0],
            )

            ps_yts += [yt_mmtile(m) for m in range(4, KT)]

            # normalize (ACT copy w/ per-partition scale) + residual + store
            for m in range(KT):
                t1 = fin_pool.tile([P, C], F32, tag="fin", name=f"t1_{s}_{m}")
                nc.scalar.activation(
                    t1[:], ps_yts[m][:], AF.Identity,
                    scale=rc_sb[:, m:m + 1],
                )
                o_sb = out_pool.tile([P, C], F32, tag="o", name=f"o_{s}_{m}")
                nc.vector.tensor_add(
                    o_sb[:], t1[:], xres_sb[:, m * C:(m + 1) * C],
                )
                nc.sync.dma_start(
                    out_d[s].rearrange("(t p) f -> t p f", p=P)[m],
                    o_sb[:],
                )


_CACHE = {}


def _build():
    if "nc" not in _CACHE:
        nc = bacc.Bacc("TRN2", target_bir_lowering=False, debug=False)
        with tile.TileContext(nc) as tc:
            _emit(tc)
        nc.compile()
        _CACHE["nc"] = nc
    return _CACHE["nc"]


def _prep_in_maps(x, W_theta, b_theta, W_phi, b_phi, W_fuse, b_fuse):
    bf = ml_dtypes.bfloat16
    scale = np.float32(1.0 / np.sqrt(np.float32(D)))
    xf = np.ascontiguousarray(np.asarray(x, dtype=np.float32).reshape(N, C, HW))
    x_bf = xf.astype(bf)
    # residual (x + b_fuse), transposed to [HW, C] to match the yT layout
    x_res = np.ascontiguousarray(
        (xf + np.asarray(b_fuse, dtype=np.float32)[None, :, None]).transpose(0, 2, 1)
    )
    wcat_t = np.ascontiguousarray(
        np.concatenate([W_theta.astype(np.float32) * scale,
                        W_phi.astype(np.float32)], axis=0).T
    ).astype(bf)
    b_cat = np.concatenate([b_theta.astype(np.float32) * scale,
                            b_phi.astype(np.float32)]).reshape(2 * D, 1)
    wfu_t = np.ascontiguousarray(W_fuse.astype(np.float32).T).astype(bf)

    in_maps = []
    for c in range(NCORES):
        sl = slice(c * NS, (c + 1) * NS)
        in_maps.append({
            "x_bf": np.ascontiguousarray(x_bf[sl]),
            "x_res": np.ascontiguousarray(x_res[sl]),
            "wcat_t": wcat_t,
            "b_cat": b_cat.astype(np.float32),
            "wfu_t": wfu_t,
        })
    return in_maps


def _run(inputs, trace=False, **kw):
    nc = _build()
    in_maps = _prep_in_maps(**inputs)
    res = bass_utils.run_bass_kernel_spmd(
        nc, in_maps, core_ids=list(range(NCORES)), trace=trace, **kw
    )
    out = np.concatenate([res.results[c]["out"] for c in range(NCORES)], axis=0)
    # device produced [NS, HW, C]; transpose back to [N, C, H, W]
    out = out.transpose(0, 2, 1).reshape(N, C, H, W)
    return np.ascontiguousarray(out, dtype=np.float32), res


def kernel(**inputs):
    out, _ = _run(inputs, trace=False)
    return out
